# revision 1
# baseline (speedup 1.0000x reference)
"""Trainium2 Bass kernel for the AI4Burgers 3x3-stencil operator.

Reference computation (per batch image, with replicate padding):
    Lu = NU*conv3x3(u, w1) - u_vel*conv3x3(u, w2) - u_vel*conv3x3(u, w3)
       = conv3x3(u, NU*w1) - u_vel * conv3x3(u, w2 + w3)

Strategy
- Data-parallel over batch: 16 images across 8 NeuronCores, 2 images/core.
  No cross-core communication.
- Per core, each image [1024, 1024] is processed in row-chunks with H on the
  SBUF partition axis and W on the free axis. The vertical (H) part of the
  stencil is a banded [K, M] stationary matrix on the TensorEngine; the
  horizontal (W) part comes from accumulating 3 matmuls whose moving operands
  are the same SBUF tile shifted by 0/1/2 columns (the tile is W+2 wide with
  replicate-padded edge columns). Band matrices are built on the host from
  the 3x3 weights (they bake in NU, w2+w3, and the H replicate-padding).
- Matmuls run as float32r (full-rate fp32 path on TRN2); accumulation is
  fp32 in PSUM. The pointwise combine (c1 - u_vel*c23) runs on the DVE.
"""

import numpy as np

NU = 0.5
B, H, W = 16, 1024, 1024
NCORES = 8
IMGS_PER_CORE = B // NCORES

# Row chunking of one 1024-row image: output rows per chunk / input rows.
# ctype 0 = top (replicate row -1 -> 0), 1 = interior, 2 = bottom.
# (ctype, K_in_rows, M_out_rows, in_row0, out_row0)
CHUNKS = (
    [(0, 127, 126, 0, 0)]
    + [(1, 128, 126, 126 * c - 1, 126 * c) for c in range(1, 8)]
    + [(2, 17, 16, 1007, 1008)]
)
NSEG = 18  # 2 convs x 3 ctypes x 3 dx band matrices, each [<=128, <=128]

_cache = {}


def _build_stationaries(w1, w2, w3):
    """Host-side band matrices: [128, 18*128] f32, segment s = conv*9 + ctype*3 + dx."""
    wa = NU * np.asarray(w1, np.float64)[0, 0]
    wb = (np.asarray(w2, np.float64) + np.asarray(w3, np.float64))[0, 0]
    stat = np.zeros((128, NSEG * 128), np.float64)
    for ci, w in enumerate((wa, wb)):
        for ctype, K, M in ((0, 127, 126), (1, 128, 126), (2, 17, 16)):
            for dx in range(3):
                s = ci * 9 + ctype * 3 + dx
                col = s * 128
                for m in range(M):
                    for dy in range(3):
                        if ctype == 0:
                            k = max(m - 1 + dy, 0)
                        elif ctype == 1:
                            k = m + dy
                        else:
                            k = min(m + dy, K - 1)
                        stat[k, col + m] += w[dy, dx]
    return stat.astype(np.float32)


def _build_program():
    from concourse import bacc, tile, mybir

    f32 = mybir.dt.float32
    f32r = mybir.dt.float32r

    nc = bacc.Bacc(None, target_bir_lowering=False, debug=False)
    u_d = nc.dram_tensor("u", [IMGS_PER_CORE, H, W], f32, kind="ExternalInput").ap()
    v_d = nc.dram_tensor("uvel", [IMGS_PER_CORE, H, W], f32, kind="ExternalInput").ap()
    s_d = nc.dram_tensor("stat", [128, NSEG * 128], f32, kind="ExternalInput").ap()
    o_d = nc.dram_tensor("out", [IMGS_PER_CORE, H, W], f32, kind="ExternalOutput").ap()

    with tile.TileContext(nc) as tc:
        with (
            tc.tile_pool(name="const", bufs=1) as cp,
            tc.tile_pool(name="up", bufs=3) as up,
            tc.tile_pool(name="vp", bufs=3) as vp,
            tc.tile_pool(name="op", bufs=3) as op,
            tc.tile_pool(name="tp", bufs=3) as tp,
            tc.tile_pool(name="pp", bufs=3, space="PSUM") as pp,
        ):
            stat_t = cp.tile([128, NSEG * 128], f32r)
            nc.sync.dma_start(stat_t[:], s_d[:].bitcast(f32r))

            for img in range(IMGS_PER_CORE):
                for ctype, K, M, rin, rout in CHUNKS:
                    ut = up.tile([128, W + 2], f32r, tag="ut")
                    nc.sync.dma_start(
                        ut[0:K, 1 : W + 1], u_d[img, rin : rin + K, :].bitcast(f32r)
                    )
                    # replicate-pad W edges
                    nc.vector.tensor_copy(ut[0:K, 0:1], ut[0:K, 1:2])
                    nc.vector.tensor_copy(ut[0:K, W + 1 : W + 2], ut[0:K, W : W + 1])

                    vt = vp.tile([128, W], f32, tag="vt")
                    nc.sync.dma_start(vt[0:M, :], v_d[img, rout : rout + M, :])

                    ot = op.tile([128, W], f32, tag="ot")
                    for hh in range(2):
                        base = 512 * hh
                        p1 = pp.tile([128, 512], f32, tag="p1")
                        p2 = pp.tile([128, 512], f32, tag="p2")
                        for conv, ps in ((0, p1), (1, p2)):
                            for dx in range(3):
                                seg = (conv * 9 + ctype * 3 + dx) * 128
                                nc.tensor.matmul(
                                    ps[0:M, :],
                                    stat_t[0:K, seg : seg + M],
                                    ut[0:K, base + dx : base + dx + 512],
                                    start=(dx == 0),
                                    stop=(dx == 2),
                                )
                        tt = tp.tile([128, 512], f32, tag="tt")
                        nc.vector.tensor_mul(
                            tt[0:M, :], vt[0:M, base : base + 512], p2[0:M, :]
                        )
                        nc.vector.tensor_sub(
                            ot[0:M, base : base + 512], p1[0:M, :], tt[0:M, :]
                        )
                    nc.sync.dma_start(o_d[img, rout : rout + M, :], ot[0:M, :])

    nc.compile()
    return nc


def _get_program():
    if "nc" not in _cache:
        _cache["nc"] = _build_program()
    return _cache["nc"]


def _make_in_maps(u, u_vel, w1, w2, w3):
    u = np.ascontiguousarray(np.asarray(u, np.float32).reshape(B, H, W))
    v = np.ascontiguousarray(np.asarray(u_vel, np.float32).reshape(B, H, W))
    stat = _build_stationaries(w1, w2, w3)
    n = IMGS_PER_CORE
    return [
        {"u": u[i * n : (i + 1) * n], "uvel": v[i * n : (i + 1) * n], "stat": stat}
        for i in range(NCORES)
    ]


def kernel(u, u_vel, w1, w2, w3):
    from concourse.bass_utils import run_bass_kernel_spmd

    nc = _get_program()
    in_maps = _make_in_maps(u, u_vel, w1, w2, w3)
    res = run_bass_kernel_spmd(nc, in_maps, core_ids=list(range(NCORES)))
    out = np.empty((B, 1, H, W), np.float32)
    n = IMGS_PER_CORE
    for i in range(NCORES):
        out[i * n : (i + 1) * n, 0] = res.results[i]["out"]
    return out


# revision 3
# speedup vs baseline: 1.2225x; 1.2225x over previous
"""Trainium2 Bass kernel for the AI4Burgers 3x3-stencil operator.

Reference computation (per batch image, with replicate padding):
    Lu = NU*conv3x3(u, w1) - u_vel*conv3x3(u, w2) - u_vel*conv3x3(u, w3)
       = conv3x3(u, NU*w1) - u_vel * conv3x3(u, w2 + w3)

Strategy
- Data-parallel over batch: 16 images across 8 NeuronCores, 2 images/core.
  No cross-core communication.
- Per core, each image [1024, 1024] is processed in row-chunks with H on the
  SBUF partition axis and W on the free axis. The vertical (H) part of the
  stencil is a banded [K, M] stationary matrix on the TensorEngine; the
  horizontal (W) part comes from accumulating 3 matmuls whose moving operands
  are the same SBUF tile shifted by 0/1/2 columns (the tile is W+2 wide with
  replicate-padded edge columns). Band matrices are built on the host from
  the 3x3 weights (they bake in NU, w2+w3, and the H replicate-padding).
- Matmuls run as float32r (full-rate fp32 path on TRN2); accumulation is
  fp32 in PSUM. The pointwise combine (c1 - u_vel*c23) runs on the DVE.
"""

import numpy as np

NU = 0.5
B, H, W = 16, 1024, 1024
NCORES = 8
IMGS_PER_CORE = B // NCORES

# Row chunking of one 1024-row image: output rows per chunk / input rows.
# ctype 0 = top (replicate row -1 -> 0), 1 = interior, 2 = bottom.
# (ctype, K_in_rows, M_out_rows, in_row0, out_row0)
CHUNKS = (
    [(0, 127, 126, 0, 0)]
    + [(1, 128, 126, 126 * c - 1, 126 * c) for c in range(1, 8)]
    + [(2, 17, 16, 1007, 1008)]
)
NSEG = 18  # 2 convs x 3 ctypes x 3 dx band matrices, each [<=128, <=128]

_cache = {}


def _build_stationaries(w1, w2, w3):
    """Host-side band matrices: [128, 18*128] f32, segment s = conv*9 + ctype*3 + dx."""
    wa = NU * np.asarray(w1, np.float64)[0, 0]
    wb = (np.asarray(w2, np.float64) + np.asarray(w3, np.float64))[0, 0]
    stat = np.zeros((128, NSEG * 128), np.float64)
    for ci, w in enumerate((wa, wb)):
        for ctype, K, M in ((0, 127, 126), (1, 128, 126), (2, 17, 16)):
            for dx in range(3):
                s = ci * 9 + ctype * 3 + dx
                col = s * 128
                for m in range(M):
                    for dy in range(3):
                        if ctype == 0:
                            k = max(m - 1 + dy, 0)
                        elif ctype == 1:
                            k = m + dy
                        else:
                            k = min(m + dy, K - 1)
                        stat[k, col + m] += w[dy, dx]
    return stat.astype(np.float32)


def _build_program():
    from concourse import bacc, tile, mybir

    f32 = mybir.dt.float32
    bf16 = mybir.dt.bfloat16

    nc = bacc.Bacc(None, target_bir_lowering=False, debug=False)
    u_d = nc.dram_tensor("u", [IMGS_PER_CORE, H, W], f32, kind="ExternalInput").ap()
    v_d = nc.dram_tensor("uvel", [IMGS_PER_CORE, H, W], f32, kind="ExternalInput").ap()
    s_d = nc.dram_tensor("stat", [128, NSEG * 128], f32, kind="ExternalInput").ap()
    o_d = nc.dram_tensor("out", [IMGS_PER_CORE, H, W], f32, kind="ExternalOutput").ap()

    with tile.TileContext(nc) as tc:
        with (
            tc.tile_pool(name="const", bufs=1) as cp,
            tc.tile_pool(name="up", bufs=4) as up,
            tc.tile_pool(name="vp", bufs=3) as vp,
            tc.tile_pool(name="op", bufs=3) as op,
            tc.tile_pool(name="tp", bufs=4) as tp,
            tc.tile_pool(name="pp", bufs=2, space="PSUM") as pp,
        ):
            stat_t = cp.tile([128, NSEG * 128], bf16)
            nc.gpsimd.dma_start(stat_t[:], s_d[:])  # f32 -> bf16 cast in DMA

            for img in range(IMGS_PER_CORE):
                for ctype, K, M, rin, rout in CHUNKS:
                    # u rows on the SWDGE ring with f32->bf16 cast in flight
                    ut = up.tile([128, W + 2], bf16, tag="ut")
                    nc.gpsimd.dma_start(
                        ut[0:K, 1 : W + 1], u_d[img, rin : rin + K, :]
                    )
                    # replicate-pad W edges
                    nc.vector.tensor_copy(ut[0:K, 0:1], ut[0:K, 1:2])
                    nc.vector.tensor_copy(ut[0:K, W + 1 : W + 2], ut[0:K, W : W + 1])

                    # u_vel on the SP HWDGE ring
                    vt = vp.tile([128, W], f32, tag="vt")
                    nc.sync.dma_start(vt[0:M, :], v_d[img, rout : rout + M, :])

                    ot = op.tile([128, W], f32, tag="ot")
                    ps = {}
                    for hh in range(2):
                        ps[0, hh] = pp.tile([128, 512], f32, tag=f"p1h{hh}", name=f"p1h{hh}")
                        ps[1, hh] = pp.tile([128, 512], f32, tag=f"p2h{hh}", name=f"p2h{hh}")
                    for conv in range(2):
                        for dx in range(3):
                            seg = (conv * 9 + ctype * 3 + dx) * 128
                            for hh in range(2):
                                base = 512 * hh
                                nc.tensor.matmul(
                                    ps[conv, hh][0:M, :],
                                    stat_t[0:K, seg : seg + M],
                                    ut[0:K, base + dx : base + dx + 512],
                                    start=(dx == 0),
                                    stop=(dx == 2),
                                )
                    for hh in range(2):
                        base = 512 * hh
                        tt = tp.tile([128, 512], f32, tag="tt")
                        nc.vector.tensor_mul(
                            tt[0:M, :], vt[0:M, base : base + 512], ps[1, hh][0:M, :]
                        )
                        nc.vector.tensor_sub(
                            ot[0:M, base : base + 512], ps[0, hh][0:M, :], tt[0:M, :]
                        )
                    # output on the ACT HWDGE ring
                    nc.scalar.dma_start(o_d[img, rout : rout + M, :], ot[0:M, :])

    nc.compile()
    return nc


def _get_program():
    if "nc" not in _cache:
        _cache["nc"] = _build_program()
    return _cache["nc"]


def _make_in_maps(u, u_vel, w1, w2, w3):
    u = np.ascontiguousarray(np.asarray(u, np.float32).reshape(B, H, W))
    v = np.ascontiguousarray(np.asarray(u_vel, np.float32).reshape(B, H, W))
    stat = _build_stationaries(w1, w2, w3)
    n = IMGS_PER_CORE
    return [
        {"u": u[i * n : (i + 1) * n], "uvel": v[i * n : (i + 1) * n], "stat": stat}
        for i in range(NCORES)
    ]


def kernel(u, u_vel, w1, w2, w3):
    from concourse.bass_utils import run_bass_kernel_spmd

    nc = _get_program()
    in_maps = _make_in_maps(u, u_vel, w1, w2, w3)
    res = run_bass_kernel_spmd(nc, in_maps, core_ids=list(range(NCORES)))
    out = np.empty((B, 1, H, W), np.float32)
    n = IMGS_PER_CORE
    for i in range(NCORES):
        out[i * n : (i + 1) * n, 0] = res.results[i]["out"]
    return out


# revision 6
# speedup vs baseline: 1.6458x; 1.3462x over previous
"""Trainium2 Bass kernel for the AI4Burgers 3x3-stencil operator.

Reference computation (per batch image, with replicate padding):
    Lu = NU*conv3x3(u, w1) - u_vel*conv3x3(u, w2) - u_vel*conv3x3(u, w3)
       = conv3x3(u, NU*w1) - u_vel * conv3x3(u, w2 + w3)

Strategy
- Data-parallel over batch: 16 images across 8 NeuronCores, 2 images/core.
  No cross-core communication.
- Per core, each image [1024, 1024] is processed in row-chunks with H on the
  SBUF partition axis and W on the free axis. The vertical (H) part of the
  stencil is a banded [K, M] stationary matrix on the TensorEngine; the
  horizontal (W) part comes from accumulating 3 matmuls whose moving operands
  are the same SBUF tile shifted by 0/1/2 columns (the tile is W+2 wide with
  replicate-padded edge columns). Band matrices are built on the host from
  the 3x3 weights (they bake in NU, w2+w3, and the H replicate-padding).
- Matmuls run as float32r (full-rate fp32 path on TRN2); accumulation is
  fp32 in PSUM. The pointwise combine (c1 - u_vel*c23) runs on the DVE.
"""

import numpy as np

NU = 0.5
B, H, W = 16, 1024, 1024
NCORES = 8
IMGS_PER_CORE = B // NCORES

# Row chunking of one 1024-row image: output rows per chunk / input rows.
# ctype 0 = top (replicate row -1 -> 0), 1 = interior, 2 = bottom.
# (ctype, K_in_rows, M_out_rows, in_row0, out_row0)
CHUNKS = (
    [(0, 127, 126, 0, 0)]
    + [(1, 128, 126, 126 * c - 1, 126 * c) for c in range(1, 8)]
    + [(2, 17, 16, 1007, 1008)]
)
NSEG = 18  # 2 convs x 3 ctypes x 3 dx band matrices, each [<=128, <=128]

_cache = {}


def _build_stationaries(w1, w2, w3):
    """Host-side band matrices: [128, 18*128] f32, segment s = conv*9 + ctype*3 + dx."""
    wa = NU * np.asarray(w1, np.float64)[0, 0]
    wb = (np.asarray(w2, np.float64) + np.asarray(w3, np.float64))[0, 0]
    stat = np.zeros((128, NSEG * 128), np.float64)
    for ci, w in enumerate((wa, wb)):
        for ctype, K, M in ((0, 127, 126), (1, 128, 126), (2, 17, 16)):
            for dx in range(3):
                s = ci * 9 + ctype * 3 + dx
                col = s * 128
                for m in range(M):
                    for dy in range(3):
                        if ctype == 0:
                            k = max(m - 1 + dy, 0)
                        elif ctype == 1:
                            k = m + dy
                        else:
                            k = min(m + dy, K - 1)
                        stat[k, col + m] += w[dy, dx]
    return stat.astype(np.float32)


def _build_program():
    from concourse import bacc, tile, mybir

    f32 = mybir.dt.float32
    bf16 = mybir.dt.bfloat16

    nc = bacc.Bacc(None, target_bir_lowering=False, debug=False)
    u_d = nc.dram_tensor("u", [IMGS_PER_CORE, H, W], bf16, kind="ExternalInput").ap()
    v_d = nc.dram_tensor("uvel", [IMGS_PER_CORE, H, W], bf16, kind="ExternalInput").ap()
    s_d = nc.dram_tensor("stat", [128, NSEG * 128], bf16, kind="ExternalInput").ap()
    o_d = nc.dram_tensor("out", [IMGS_PER_CORE, H, W], bf16, kind="ExternalOutput").ap()

    with tile.TileContext(nc) as tc:
        with (
            tc.tile_pool(name="const", bufs=1) as cp,
            tc.tile_pool(name="up", bufs=4) as up,
            tc.tile_pool(name="vp", bufs=4) as vp,
            tc.tile_pool(name="op", bufs=4) as op,
            tc.tile_pool(name="tp", bufs=4) as tp,
            tc.tile_pool(name="pp", bufs=2, space="PSUM") as pp,
        ):
            stat_t = cp.tile([128, NSEG * 128], bf16)
            nc.sync.dma_start(stat_t[:], s_d[:])

            for img in range(IMGS_PER_CORE):
                for ctype, K, M, rin, rout in CHUNKS:
                    # u rows on the SP HWDGE ring
                    ut = up.tile([128, W + 2], bf16, tag="ut")
                    nc.sync.dma_start(
                        ut[0:K, 1 : W + 1], u_d[img, rin : rin + K, :]
                    )
                    # replicate-pad W edges
                    nc.vector.tensor_copy(ut[0:K, 0:1], ut[0:K, 1:2])
                    nc.vector.tensor_copy(ut[0:K, W + 1 : W + 2], ut[0:K, W : W + 1])

                    # u_vel on the ACT HWDGE ring
                    vt = vp.tile([128, W], bf16, tag="vt")
                    nc.scalar.dma_start(vt[0:M, :], v_d[img, rout : rout + M, :])

                    ot = op.tile([128, W], bf16, tag="ot")
                    ps = {}
                    for hh in range(2):
                        ps[0, hh] = pp.tile([128, 512], f32, tag=f"p1h{hh}", name=f"p1h{hh}")
                        ps[1, hh] = pp.tile([128, 512], f32, tag=f"p2h{hh}", name=f"p2h{hh}")
                    for conv in range(2):
                        for dx in range(3):
                            seg = (conv * 9 + ctype * 3 + dx) * 128
                            for hh in range(2):
                                base = 512 * hh
                                nc.tensor.matmul(
                                    ps[conv, hh][0:M, :],
                                    stat_t[0:K, seg : seg + M],
                                    ut[0:K, base + dx : base + dx + 512],
                                    start=(dx == 0),
                                    stop=(dx == 2),
                                )
                    for hh in range(2):
                        base = 512 * hh
                        tt = tp.tile([128, 512], f32, tag="tt")
                        nc.vector.tensor_mul(
                            tt[0:M, :], vt[0:M, base : base + 512], ps[1, hh][0:M, :]
                        )
                        nc.vector.tensor_sub(
                            ot[0:M, base : base + 512], ps[0, hh][0:M, :], tt[0:M, :]
                        )
                    # output on the Pool SWDGE ring
                    nc.gpsimd.dma_start(o_d[img, rout : rout + M, :], ot[0:M, :])

    nc.compile()
    return nc


def _get_program():
    if "nc" not in _cache:
        _cache["nc"] = _build_program()
    return _cache["nc"]


def _make_in_maps(u, u_vel, w1, w2, w3):
    import ml_dtypes

    bf = ml_dtypes.bfloat16
    u = np.ascontiguousarray(np.asarray(u, np.float32).reshape(B, H, W).astype(bf))
    v = np.ascontiguousarray(np.asarray(u_vel, np.float32).reshape(B, H, W).astype(bf))
    stat = _build_stationaries(w1, w2, w3).astype(bf)
    n = IMGS_PER_CORE
    return [
        {"u": u[i * n : (i + 1) * n], "uvel": v[i * n : (i + 1) * n], "stat": stat}
        for i in range(NCORES)
    ]


def kernel(u, u_vel, w1, w2, w3):
    from concourse.bass_utils import run_bass_kernel_spmd

    nc = _get_program()
    in_maps = _make_in_maps(u, u_vel, w1, w2, w3)
    res = run_bass_kernel_spmd(nc, in_maps, core_ids=list(range(NCORES)))
    out = np.empty((B, 1, H, W), np.float32)
    n = IMGS_PER_CORE
    for i in range(NCORES):
        out[i * n : (i + 1) * n, 0] = res.results[i]["out"].astype(np.float32)
    return out
